# revision 10
# baseline (speedup 1.0000x reference)
"""Trainium2 Bass kernel for the AF-2D-MADE autoregressive sampling block.

Strategy:
- Data-parallel over batch: 16 samples -> 8 NeuronCores x 2 samples, no
  collectives; host shards inputs and concatenates outputs.
- Wavefront scheduling: pixels with equal t = 2i + j are independent (the
  masked-conv receptive field at (i,j) only reaches row i-r up to column j+r,
  and column j-1 within row i), so the 64-pixel raster scan collapses to 22
  sequential wavefront steps updating up to 4 pixels each.
- Both conv networks (mu, lv) are fused into single matmuls with
  block-diagonal weights (64+64 channels on the 128-partition contraction).
- Convs are implicit GEMMs over mask taps: activations live in SBUF as
  [chan, B*(10*10)] zero/one-padded images so each tap is a strided AP read.
- ELU is computed in the u = elu(h)+1 representation:
      u = max(h + c + 1, min(exp(h + c), 1))
  (exact since exp(x) >= x+1, and max|h| ~ 3 so exp never overflows), with
  pad ring = 1.0 and the -sum(W) bias corrections c folded in, so each stage
  is 1 ACT op + 2 DVE ops.
"""

import numpy as np
from contextlib import ExitStack

import concourse.bacc as bacc
import concourse.bass as bass
import concourse.mybir as mybir
import concourse.tile as tile
from concourse.bass_utils import run_bass_kernel_spmd

N_CORES = 8
BL = 2  # batch per core
F32 = mybir.dt.float32
AF = mybir.ActivationFunctionType
ALU = mybir.AluOpType
TAPS_A = [(0, 0), (0, 1), (0, 2), (1, 0)]
TAPS_B = [(0, 0), (0, 1), (0, 2), (1, 0), (1, 1)]

TRACE = False
LAST_RESULT = None
_NC_CACHE = None

PARAM_SHAPES = [
    ("x_adj", [3, 128]),
    ("w0", [3, 512]),
    ("w1", [128, 640]),
    ("w2", [128, 640]),
    ("w3", [128, 175]),
    ("c1", [128, 1]),
    ("c2", [128, 1]),
    ("c3", [128, 1]),
    ("c4lv", [3, 1]),
    ("nc4lvh", [3, 1]),
]


def _img(ap):
    """[P, 200] -> [P, h, (w b)] padded-image view; layout is (h, w, b)."""
    return ap.rearrange("p (h wb) -> p h wb", h=10, wb=10 * BL)


def _qb(ap):
    """[P, n*BL] -> [P, q, b] view (b innermost)."""
    n = ap.shape[-1] // BL
    return ap.rearrange("p (q b) -> p q b", b=BL, q=n)


def build_nc():
    nc = bacc.Bacc("TRN2", debug=False, num_devices=N_CORES)
    prm = {}
    for name, shape in PARAM_SHAPES:
        prm[name] = nc.declare_dram_parameter(name, shape, F32, isOutput=False)
    out_y = nc.declare_dram_parameter("out_y", [3, 128], F32, isOutput=True)
    out_ls = nc.declare_dram_parameter("out_ls", [1, BL], F32, isOutput=True)

    with ExitStack() as ctx:
        tc = ctx.enter_context(tile.TileContext(nc))
        const = ctx.enter_context(tc.tile_pool(name="const", bufs=1))
        state = ctx.enter_context(tc.tile_pool(name="state", bufs=1))
        tmp = ctx.enter_context(tc.tile_pool(name="tmp", bufs=3))
        psum = ctx.enter_context(tc.tile_pool(name="psum", bufs=1, space="PSUM"))

        # --- load params ---
        sb = {}
        for name, shape in PARAM_SHAPES:
            sb[name] = const.tile(shape, F32, tag=name, name=f"sb_{name}")
            nc.sync.dma_start(sb[name][:], prm[name][:])

        # --- persistent state ---
        y_pad = state.tile([3, BL * 100], F32, tag="y_pad")
        nc.gpsimd.memset(y_pad[:], 0.0)
        us = []
        for l in range(3):
            u = state.tile([128, BL * 100], F32, tag=f"u{l + 1}")
            nc.gpsimd.memset(u[:], 1.0)
            us.append(u)
        lsbuf = state.tile([3, BL * 64], F32, tag="lsbuf")
        nc.gpsimd.memset(lsbuf[:], 0.0)

        def conv(src, wt, taps, m_out, ptag):
            """src [P,200] padded; wt [K, ntaps*m_out]; returns PSUM [m_out, 128]."""
            h = psum.tile([m_out, BL * 64], F32, tag=ptag, bufs=2 if m_out == 128 else 1)
            for k, (ky, kx) in enumerate(taps):
                rhs = _img(src[:])[:, ky : ky + 8, BL * kx : BL * (kx + 8)]
                nc.tensor.matmul(
                    h[:],
                    wt[:, k * m_out : (k + 1) * m_out],
                    rhs,
                    start=(k == 0),
                    stop=(k == len(taps) - 1),
                )
            return h

        def elu_stage(h, c_t, u_out):
            """u_out interior <- elu(h + c) + 1 ; h is PSUM [128,128]."""
            ex = tmp.tile([128, BL * 64], F32, tag="ex")
            nc.scalar.activation(ex[:], h[:], AF.Exp, bias=c_t[:], scale=1.0)
            tp = tmp.tile([128, BL * 64], F32, tag="tp")
            nc.vector.tensor_scalar(tp[:], h[:], c_t[:], 1.0, ALU.add, ALU.add)
            u_int = _img(u_out[:])[:, 1:9, BL : 9 * BL]
            nc.vector.scalar_tensor_tensor(u_int, ex[:], 1.0, tp[:], ALU.min, ALU.max)

        # --- 22 wavefront steps ---
        for t in range(22):
            i_min = max(0, (t - 6) // 2)
            i_max = min(7, t // 2)
            n_i = i_max - i_min + 1

            h1 = conv(y_pad, sb["w0"][:], TAPS_A, 128, "h1")
            elu_stage(h1, sb["c1"], us[0])
            h2 = conv(us[0], sb["w1"][:], TAPS_B, 128, "h2")
            elu_stage(h2, sb["c2"], us[1])
            h3 = conv(us[1], sb["w2"][:], TAPS_B, 128, "h3")
            elu_stage(h3, sb["c3"], us[2])
            o = conv(us[2], sb["w3"][:], TAPS_B, 35, "o")

            qbase = t + 6 * i_min
            sl = slice(qbase, qbase + 6 * (n_i - 1) + 1, 6)
            o_mu = _qb(o[0:3, :])[:, sl, :]
            o_lv = _qb(o[32:35, :])[:, sl, :]

            # rinv = exp(-0.5*(lv + c4lv)) = 1/(exp(logstd))
            rinv = tmp.tile([3, BL * n_i], F32, tag="rinv")
            nc.scalar.activation(rinv[:], o_lv, AF.Exp, bias=sb["nc4lvh"][:], scale=-0.5)
            # num = x_adj - mu_psum  (x_adj already has -c4mu folded in)
            num = tmp.tile([3, BL * n_i], F32, tag="num")
            xc = _qb(sb["x_adj"][:])[:, sl, :]
            nc.vector.scalar_tensor_tensor(num[:], o_mu, -1.0, xc, ALU.mult, ALU.add)
            # y[wavefront] = num * rinv
            ybase = 11 + t + 8 * i_min
            ywf = _qb(y_pad[:])[:, ybase : ybase + 8 * (n_i - 1) + 1 : 8, :]
            nvw = num[:].rearrange("p (q b) -> p q b", b=BL, q=n_i)
            rvw = rinv[:].rearrange("p (q b) -> p q b", b=BL, q=n_i)
            nc.vector.tensor_tensor(ywf, nvw, rvw, ALU.mult)
            # logstd[wavefront] = 0.5*(lv + c4lv)
            lwf = _qb(lsbuf[:])[:, sl, :]
            nc.vector.tensor_scalar(lwf, o_lv, sb["c4lv"][:], 0.5, ALU.add, ALU.mult)

        # --- epilogue: lsum + outputs ---
        red = state.tile([3, BL], F32, tag="red")
        lsv = lsbuf[:].rearrange("p (q b) -> p b q", b=BL, q=64)
        nc.vector.tensor_reduce(red[:], lsv, mybir.AxisListType.X, ALU.add)
        ones3 = state.tile([3, 1], F32, tag="ones3")
        nc.gpsimd.memset(ones3[:], 1.0)
        lsps = psum.tile([1, BL], F32, tag="lsps")
        nc.tensor.matmul(lsps[:], ones3[:], red[:], start=True, stop=True)
        lso = state.tile([1, BL], F32, tag="lso")
        nc.vector.tensor_copy(lso[:], lsps[:])
        nc.sync.dma_start(out_ls[:], lso[:])
        y_int = _img(y_pad[:])[:, 1:9, BL : 9 * BL]
        nc.sync.dma_start(out_y[:], y_int)

    nc.compile()
    return nc


def prep_params(inputs):
    """Host-side preprocessing of weights (shared across cores)."""
    g = {k: np.asarray(v, np.float32) for k, v in inputs.items()}

    def bd(a, b):
        out = np.zeros((a.shape[0] + b.shape[0], a.shape[1] + b.shape[1]), np.float32)
        out[: a.shape[0], : a.shape[1]] = a
        out[a.shape[0] :, a.shape[1] :] = b
        return out

    p = {}
    p["w0"] = np.ascontiguousarray(
        np.concatenate(
            [
                np.concatenate(
                    [g["mu_w0"][:, :, ky, kx].T, g["lv_w0"][:, :, ky, kx].T], axis=1
                )
                for ky, kx in TAPS_A
            ],
            axis=1,
        )
    )
    for l, name in ((1, "w1"), (2, "w2")):
        p[name] = np.ascontiguousarray(
            np.concatenate(
                [
                    bd(g[f"mu_w{l}"][:, :, ky, kx].T, g[f"lv_w{l}"][:, :, ky, kx].T)
                    for ky, kx in TAPS_B
                ],
                axis=1,
            )
        )

    def bd35(a, b):
        # mu cols 0-2, lv cols 32-34 (32-aligned partition base for PSUM reads)
        out = np.zeros((128, 35), np.float32)
        out[:64, 0:3] = a
        out[64:, 32:35] = b
        return out

    p["w3"] = np.ascontiguousarray(
        np.concatenate(
            [
                bd35(g["mu_w3"][:, :, ky, kx].T, g["lv_w3"][:, :, ky, kx].T)
                for ky, kx in TAPS_B
            ],
            axis=1,
        )
    )
    p["c1"] = np.concatenate([g["mu_b0"], g["lv_b0"]]).reshape(128, 1)
    for l, name in ((1, "c2"), (2, "c3")):
        p[name] = np.concatenate(
            [
                g[f"mu_b{l}"]
                - sum(g[f"mu_w{l}"][:, :, ky, kx].sum(1) for ky, kx in TAPS_B),
                g[f"lv_b{l}"]
                - sum(g[f"lv_w{l}"][:, :, ky, kx].sum(1) for ky, kx in TAPS_B),
            ]
        ).reshape(128, 1)
    c4mu = g["mu_b3"] - sum(g["mu_w3"][:, :, ky, kx].sum(1) for ky, kx in TAPS_B)
    c4lv = g["lv_b3"] - sum(g["lv_w3"][:, :, ky, kx].sum(1) for ky, kx in TAPS_B)
    p["c4lv"] = np.ascontiguousarray(c4lv.reshape(3, 1))
    p["nc4lvh"] = np.ascontiguousarray((-0.5 * c4lv).reshape(3, 1))
    p["_c4mu"] = c4mu
    return p


def make_in_maps(inputs):
    p = prep_params(inputs)
    x = np.asarray(inputs["x"], np.float32)
    shared = {k: v for k, v in p.items() if not k.startswith("_")}
    in_maps = []
    for c in range(N_CORES):
        xs = x[c * BL : (c + 1) * BL]
        # layout (c, h, w, b)
        x_adj = xs.transpose(1, 2, 3, 0).reshape(3, 64 * BL) - p["_c4mu"][:, None]
        m = dict(shared)
        m["x_adj"] = np.ascontiguousarray(x_adj, np.float32)
        in_maps.append(m)
    return in_maps


def kernel(**inputs):
    global _NC_CACHE, LAST_RESULT
    if _NC_CACHE is None:
        _NC_CACHE = build_nc()
    in_maps = make_in_maps(inputs)
    res = run_bass_kernel_spmd(
        _NC_CACHE, in_maps, core_ids=list(range(N_CORES)), trace=TRACE
    )
    LAST_RESULT = res
    ys, lss = [], []
    for c in range(N_CORES):
        ys.append(res.results[c]["out_y"].reshape(3, 8, 8, BL).transpose(3, 0, 1, 2))
        lss.append(res.results[c]["out_ls"].reshape(BL))
    return (
        np.ascontiguousarray(np.concatenate(ys), dtype=np.float32),
        np.ascontiguousarray(np.concatenate(lss), dtype=np.float32),
    )


# revision 12
# speedup vs baseline: 2.0241x; 2.0241x over previous
"""Trainium2 Bass kernel for the AF-2D-MADE autoregressive sampling block.

Strategy:
- Data-parallel over batch: 16 samples -> 8 NeuronCores x 2 samples, no
  collectives; host shards inputs and concatenates outputs.
- Wavefront scheduling: pixels with equal t = 2i + j are independent (the
  masked-conv receptive field at (i,j) only reaches row i-r up to column j+r,
  and column j-1 within row i), so the 64-pixel raster scan collapses to 22
  sequential wavefront steps updating up to 4 pixels each.
- Both conv networks (mu, lv) are fused into single matmuls with
  block-diagonal weights (64+64 channels on the 128-partition contraction).
- Convs are implicit GEMMs over mask taps: activations live in SBUF as
  [chan, (10, 10, B)] zero/one-padded images so each tap is a strided AP read.
- ELU is computed in the u = elu(h)+1 representation:
      u = max(h + c + 1, min(exp(h + c), 1))
  (exact since exp(x) >= x+1, and |h| ~ 3 so exp never overflows), with
  pad ring = 1.0 and the -sum(W) bias corrections c folded in, so each stage
  is 1 ACT op + 2 DVE ops.
- Matmuls run in fp16 (fp32 PE matmul is ~4x slower: no FWL + half-rate
  streaming). The logstd SUM is cancellation-sensitive (192 correlated
  terms), so it is recomputed at the end by a one-time fp32 lv-net replay
  over the final y. (Valid because no lv output depends on y[7,7]: the
  masks exclude self/raster-later pixels, so the replay on fully-updated y
  equals the reference's final-step logstd exactly.)
"""

import numpy as np
from contextlib import ExitStack

import concourse.bacc as bacc
import concourse.bass as bass
import concourse.mybir as mybir
import concourse.tile as tile
from concourse.bass_utils import run_bass_kernel_spmd

N_CORES = 8
BL = 2  # batch per core
F32 = mybir.dt.float32
F16 = mybir.dt.float16
AF = mybir.ActivationFunctionType
ALU = mybir.AluOpType
TAPS_A = [(0, 0), (0, 1), (0, 2), (1, 0)]
TAPS_B = [(0, 0), (0, 1), (0, 2), (1, 0), (1, 1)]

TRACE = False
LAST_RESULT = None
_NC_CACHE = None

PARAM_SHAPES = [
    # fp16 fused loop weights (mu | lv block-diagonal per tap)
    ("w0", [3, 4 * 128], F16),
    ("w1", [128, 5 * 128], F16),
    ("w2", [128, 5 * 128], F16),
    ("w3", [128, 5 * 35], F16),
    # fp32 lv-only replay weights
    ("v0", [3, 4 * 64], F32),
    ("v1", [64, 5 * 64], F32),
    ("v2", [64, 5 * 64], F32),
    ("v3", [64, 5 * 3], F32),
    # bias-correction vectors
    ("c1", [128, 1], F32),
    ("c2", [128, 1], F32),
    ("c3", [128, 1], F32),
    ("c1l", [64, 1], F32),
    ("c2l", [64, 1], F32),
    ("c3l", [64, 1], F32),
    ("c4lv", [3, 1], F32),
    ("nc4lvh", [3, 1], F32),
    # per-core data
    ("x_adj", [3, 8 * 8 * BL], F32),
]


def _img(ap):
    """[P, 200] -> [P, h, (w b)] padded-image view; layout is (h, w, b)."""
    return ap.rearrange("p (h wb) -> p h wb", h=10, wb=10 * BL)


def _qb(ap):
    """[P, n*BL] -> [P, q, b] view (b innermost)."""
    n = ap.shape[-1] // BL
    return ap.rearrange("p (q b) -> p q b", b=BL, q=n)


def build_nc():
    nc = bacc.Bacc("TRN2", debug=False, num_devices=N_CORES)
    prm = {}
    for name, shape, dt in PARAM_SHAPES:
        prm[name] = nc.declare_dram_parameter(name, shape, dt, isOutput=False)
    out_y = nc.declare_dram_parameter("out_y", [3, 64 * BL], F32, isOutput=True)
    out_ls = nc.declare_dram_parameter("out_ls", [1, BL], F32, isOutput=True)

    with ExitStack() as ctx:
        tc = ctx.enter_context(tile.TileContext(nc))
        const = ctx.enter_context(tc.tile_pool(name="const", bufs=1))
        state = ctx.enter_context(tc.tile_pool(name="state", bufs=1))
        tmp = ctx.enter_context(tc.tile_pool(name="tmp", bufs=3))
        psum = ctx.enter_context(tc.tile_pool(name="psum", bufs=1, space="PSUM"))

        # --- load params ---
        sb = {}
        for name, shape, dt in PARAM_SHAPES:
            sb[name] = const.tile(shape, dt, tag=name, name=f"sb_{name}")
            nc.sync.dma_start(sb[name][:], prm[name][:])

        # --- persistent state ---
        y16 = state.tile([3, BL * 100], F16, tag="y16")
        nc.gpsimd.memset(y16[:], 0.0)
        us = []
        for l in range(3):
            u = state.tile([128, BL * 100], F16, tag=f"u{l + 1}", name=f"u{l + 1}")
            nc.gpsimd.memset(u[:], 1.0)
            us.append(u)

        def conv(src, wt, taps, m_out, ptag, pbufs=1):
            """src [K,200] padded; wt [K, ntaps*m_out]; returns PSUM [m_out, 128]."""
            h = psum.tile([m_out, BL * 64], F32, tag=ptag, bufs=pbufs, name=ptag)
            for k, (ky, kx) in enumerate(taps):
                rhs = _img(src[:])[:, ky : ky + 8, BL * kx : BL * (kx + 8)]
                nc.tensor.matmul(
                    h[:],
                    wt[:, k * m_out : (k + 1) * m_out],
                    rhs,
                    start=(k == 0),
                    stop=(k == len(taps) - 1),
                )
            return h

        def elu_stage(h, c_t, u_out, pfx):
            """u_out interior <- elu(h + c) + 1 ; h is PSUM [P, 128]."""
            p = h.shape[0]
            ex = tmp.tile([p, BL * 64], F32, tag=f"{pfx}ex", name=f"{pfx}ex")
            nc.scalar.activation(ex[:], h[:], AF.Exp, bias=c_t[:], scale=1.0)
            tp = tmp.tile([p, BL * 64], F32, tag=f"{pfx}tp", name=f"{pfx}tp")
            nc.vector.tensor_scalar(tp[:], h[:], c_t[:], 1.0, ALU.add, ALU.add)
            u_int = _img(u_out[:])[:, 1:9, BL : 9 * BL]
            nc.vector.scalar_tensor_tensor(u_int, ex[:], 1.0, tp[:], ALU.min, ALU.max)

        # --- 22 wavefront steps (all-fp16 matmuls) ---
        for t in range(22):
            i_min = max(0, (t - 6) // 2)
            i_max = min(7, t // 2)
            n_i = i_max - i_min + 1

            h1 = conv(y16, sb["w0"][:], TAPS_A, 128, "h1", pbufs=2)
            elu_stage(h1, sb["c1"], us[0], "a")
            h2 = conv(us[0], sb["w1"][:], TAPS_B, 128, "h2", pbufs=2)
            elu_stage(h2, sb["c2"], us[1], "a")
            h3 = conv(us[1], sb["w2"][:], TAPS_B, 128, "h3", pbufs=2)
            elu_stage(h3, sb["c3"], us[2], "a")
            o = conv(us[2], sb["w3"][:], TAPS_B, 35, "o")

            qbase = t + 6 * i_min
            sl = slice(qbase, qbase + 6 * (n_i - 1) + 1, 6)
            o_mu = _qb(o[0:3, :])[:, sl, :]
            o_lv = _qb(o[32:35, :])[:, sl, :]

            # rinv = exp(-0.5*(lv + c4lv)) = 1/exp(logstd)
            rinv = tmp.tile([3, BL * n_i], F32, tag="rinv", name="rinv")
            nc.scalar.activation(rinv[:], o_lv, AF.Exp, bias=sb["nc4lvh"][:], scale=-0.5)
            # num = x_adj - mu_psum  (x_adj already has -c4mu folded in)
            num = tmp.tile([3, BL * n_i], F32, tag="num", name="num")
            xc = _qb(sb["x_adj"][:])[:, sl, :]
            nc.vector.scalar_tensor_tensor(num[:], o_mu, -1.0, xc, ALU.mult, ALU.add)
            # y16[wavefront] = num * rinv
            ybase = 11 + t + 8 * i_min
            ywf = _qb(y16[:])[:, ybase : ybase + 8 * (n_i - 1) + 1 : 8, :]
            nvw = num[:].rearrange("p (q b) -> p q b", b=BL, q=n_i)
            rvw = rinv[:].rearrange("p (q b) -> p q b", b=BL, q=n_i)
            nc.vector.tensor_tensor(ywf, nvw, rvw, ALU.mult)

        # --- epilogue 1: fp32 y copy + y output DMA ---
        y32 = state.tile([3, BL * 100], F32, tag="y32")
        nc.vector.tensor_copy(y32[:], y16[:])
        y_int = _img(y32[:])[:, 1:9, BL : 9 * BL]
        nc.sync.dma_start(out_y[:], y_int)

        # --- epilogue 2: fp32 lv-only replay for the logstd sum ---
        ru = []
        for l in range(3):
            u = state.tile([64, BL * 100], F32, tag=f"ru{l + 1}", name=f"ru{l + 1}")
            nc.gpsimd.memset(u[:], 1.0)
            ru.append(u)
        r1 = conv(y32, sb["v0"][:], TAPS_A, 64, "h1", pbufs=2)
        elu_stage(r1, sb["c1l"], ru[0], "r")
        r2 = conv(ru[0], sb["v1"][:], TAPS_B, 64, "h2", pbufs=2)
        elu_stage(r2, sb["c2l"], ru[1], "r")
        r3 = conv(ru[1], sb["v2"][:], TAPS_B, 64, "h3", pbufs=2)
        elu_stage(r3, sb["c3l"], ru[2], "r")
        olv = conv(ru[2], sb["v3"][:], TAPS_B, 3, "o")
        lsbuf = state.tile([3, BL * 64], F32, tag="lsbuf")
        nc.vector.tensor_scalar(lsbuf[:], olv[:], sb["c4lv"][:], 0.5, ALU.add, ALU.mult)
        red = state.tile([3, BL], F32, tag="red")
        lsv = lsbuf[:].rearrange("p (q b) -> p b q", b=BL, q=64)
        nc.vector.tensor_reduce(red[:], lsv, mybir.AxisListType.X, ALU.add)
        ones3 = state.tile([3, 1], F32, tag="ones3")
        nc.gpsimd.memset(ones3[:], 1.0)
        lsps = psum.tile([1, BL], F32, tag="lsps")
        nc.tensor.matmul(lsps[:], ones3[:], red[:], start=True, stop=True)
        lso = state.tile([1, BL], F32, tag="lso")
        nc.vector.tensor_copy(lso[:], lsps[:])
        nc.sync.dma_start(out_ls[:], lso[:])

    nc.compile()
    return nc


def prep_params(inputs):
    """Host-side preprocessing of weights (shared across cores)."""
    g = {k: np.asarray(v, np.float32) for k, v in inputs.items()}

    def bd(a, b):
        out = np.zeros((a.shape[0] + b.shape[0], a.shape[1] + b.shape[1]), np.float32)
        out[: a.shape[0], : a.shape[1]] = a
        out[a.shape[0] :, a.shape[1] :] = b
        return out

    def bd35(a, b):
        # mu cols 0-2, lv cols 32-34 (32-aligned partition base for PSUM reads)
        out = np.zeros((128, 35), np.float32)
        out[:64, 0:3] = a
        out[64:, 32:35] = b
        return out

    p = {}
    p["w0"] = np.concatenate(
        [
            np.concatenate(
                [g["mu_w0"][:, :, ky, kx].T, g["lv_w0"][:, :, ky, kx].T], axis=1
            )
            for ky, kx in TAPS_A
        ],
        axis=1,
    ).astype(np.float16)
    for l, name in ((1, "w1"), (2, "w2")):
        p[name] = np.concatenate(
            [
                bd(g[f"mu_w{l}"][:, :, ky, kx].T, g[f"lv_w{l}"][:, :, ky, kx].T)
                for ky, kx in TAPS_B
            ],
            axis=1,
        ).astype(np.float16)
    p["w3"] = np.concatenate(
        [
            bd35(g["mu_w3"][:, :, ky, kx].T, g["lv_w3"][:, :, ky, kx].T)
            for ky, kx in TAPS_B
        ],
        axis=1,
    ).astype(np.float16)
    # fp32 lv-only replay weights
    p["v0"] = np.ascontiguousarray(
        np.concatenate([g["lv_w0"][:, :, ky, kx].T for ky, kx in TAPS_A], axis=1)
    )
    for l, name in ((1, "v1"), (2, "v2"), (3, "v3")):
        p[name] = np.ascontiguousarray(
            np.concatenate([g[f"lv_w{l}"][:, :, ky, kx].T for ky, kx in TAPS_B], axis=1)
        )
    # bias corrections: layer l>=1 input is u-1 with u-pad=1 -> c_l = b_l - sum(W_l)
    p["c1"] = np.concatenate([g["mu_b0"], g["lv_b0"]]).reshape(128, 1)
    for l, name in ((1, "c2"), (2, "c3")):
        p[name] = np.concatenate(
            [
                g[f"mu_b{l}"]
                - sum(g[f"mu_w{l}"][:, :, ky, kx].sum(1) for ky, kx in TAPS_B),
                g[f"lv_b{l}"]
                - sum(g[f"lv_w{l}"][:, :, ky, kx].sum(1) for ky, kx in TAPS_B),
            ]
        ).reshape(128, 1)
    p["c1l"] = np.ascontiguousarray(p["c1"][64:])
    p["c2l"] = np.ascontiguousarray(p["c2"][64:])
    p["c3l"] = np.ascontiguousarray(p["c3"][64:])
    c4mu = g["mu_b3"] - sum(g["mu_w3"][:, :, ky, kx].sum(1) for ky, kx in TAPS_B)
    c4lv = g["lv_b3"] - sum(g["lv_w3"][:, :, ky, kx].sum(1) for ky, kx in TAPS_B)
    p["c4lv"] = np.ascontiguousarray(c4lv.reshape(3, 1))
    p["nc4lvh"] = np.ascontiguousarray((-0.5 * c4lv).reshape(3, 1))
    p["_c4mu"] = c4mu
    return p


def make_in_maps(inputs):
    p = prep_params(inputs)
    x = np.asarray(inputs["x"], np.float32)
    shared = {k: np.ascontiguousarray(v) for k, v in p.items() if not k.startswith("_")}
    in_maps = []
    for c in range(N_CORES):
        xs = x[c * BL : (c + 1) * BL]
        # layout (c, h, w, b)
        x_adj = xs.transpose(1, 2, 3, 0).reshape(3, 64 * BL) - p["_c4mu"][:, None]
        m = dict(shared)
        m["x_adj"] = np.ascontiguousarray(x_adj, np.float32)
        in_maps.append(m)
    return in_maps


def kernel(**inputs):
    global _NC_CACHE, LAST_RESULT
    if _NC_CACHE is None:
        _NC_CACHE = build_nc()
    in_maps = make_in_maps(inputs)
    res = run_bass_kernel_spmd(
        _NC_CACHE, in_maps, core_ids=list(range(N_CORES)), trace=TRACE
    )
    LAST_RESULT = res
    ys, lss = [], []
    for c in range(N_CORES):
        ys.append(res.results[c]["out_y"].reshape(3, 8, 8, BL).transpose(3, 0, 1, 2))
        lss.append(res.results[c]["out_ls"].reshape(BL))
    return (
        np.ascontiguousarray(np.concatenate(ys), dtype=np.float32),
        np.ascontiguousarray(np.concatenate(lss), dtype=np.float32),
    )


# revision 14
# speedup vs baseline: 3.7587x; 1.8570x over previous
"""Trainium2 Bass kernel for the AF-2D-MADE autoregressive sampling block.

Strategy:
- Data-parallel over batch: 16 samples -> 8 NeuronCores x 2 samples, no
  collectives; host shards inputs and concatenates outputs.
- Wavefront scheduling: pixels with equal t = 2i + j are independent (the
  masked-conv receptive field at (i,j) only reaches row i-r up to column j+r,
  and column j-1 within row i), so the 64-pixel raster scan collapses to 22
  sequential wavefront steps updating up to 4 pixels each.
- Both conv networks (mu, lv) are fused into single matmuls with
  block-diagonal weights (64+64 channels on the 128-partition contraction).
- Convs are implicit GEMMs over mask taps: activations live in SBUF as
  [chan, (10, 10, B)] zero/one-padded images so each tap is a strided AP read.
- ELU is computed in the u = elu(h)+1 representation:
      u = max(h + c + 1, min(exp(h + c), 1))
  (exact since exp(x) >= x+1, and |h| ~ 3 so exp never overflows), with
  pad ring = 1.0 and the -sum(W) bias corrections c folded in, so each stage
  is 1 ACT op + 2 DVE ops.
- Matmuls run in fp16 (fp32 PE matmul is ~4x slower: no FWL + half-rate
  streaming). The logstd SUM is cancellation-sensitive (192 correlated
  terms), so it is recomputed at the end by a one-time fp32 lv-net replay
  over the final y. (Valid because no lv output depends on y[7,7]: the
  masks exclude self/raster-later pixels, so the replay on fully-updated y
  equals the reference's final-step logstd exactly.)
"""

import numpy as np
from contextlib import ExitStack

import concourse.bacc as bacc
import concourse.bass as bass
import concourse.mybir as mybir
import concourse.tile as tile
from concourse.bass_utils import run_bass_kernel_spmd

N_CORES = 8
BL = 2  # batch per core
F32 = mybir.dt.float32
F16 = mybir.dt.float16
AF = mybir.ActivationFunctionType
ALU = mybir.AluOpType
TAPS_A = [(0, 0), (0, 1), (0, 2), (1, 0)]
TAPS_B = [(0, 0), (0, 1), (0, 2), (1, 0), (1, 1)]

TRACE = False
LAST_RESULT = None
_NC_CACHE = None

PARAM_SHAPES = [
    # fp16 fused loop weights (mu | lv block-diagonal per tap)
    ("w0", [3, 4 * 128], F16),
    ("w1", [128, 5 * 128], F16),
    ("w2", [128, 5 * 128], F16),
    ("w3m", [128, 5 * 3], F16),
    ("w3l", [128, 5 * 3], F16),
    # (c+1) rows for the const-tap PSUM bias fold (K=1 matmul)
    ("d1", [1, 128], F16),
    ("d2", [1, 128], F16),
    ("d3", [1, 128], F16),
    # fp32 lv-only replay weights
    ("v0", [3, 4 * 64], F32),
    ("v1", [64, 5 * 64], F32),
    ("v2", [64, 5 * 64], F32),
    ("v3", [64, 5 * 3], F32),
    # bias-correction vectors
    ("c1l", [64, 1], F32),
    ("c2l", [64, 1], F32),
    ("c3l", [64, 1], F32),
    ("c4lv", [3, 1], F32),
    ("nc4lvh", [3, 1], F32),
    # per-core data
    ("x_adj", [3, 8 * 8 * BL], F32),
]


def _img(ap):
    """[P, 200] -> [P, h, (w b)] padded-image view; layout is (h, w, b)."""
    return ap.rearrange("p (h wb) -> p h wb", h=10, wb=10 * BL)


def _qb(ap):
    """[P, n*BL] -> [P, q, b] view (b innermost)."""
    n = ap.shape[-1] // BL
    return ap.rearrange("p (q b) -> p q b", b=BL, q=n)


def build_nc():
    nc = bacc.Bacc("TRN2", debug=False, num_devices=N_CORES)
    prm = {}
    for name, shape, dt in PARAM_SHAPES:
        prm[name] = nc.declare_dram_parameter(name, shape, dt, isOutput=False)
    out_y = nc.declare_dram_parameter("out_y", [3, 64 * BL], F32, isOutput=True)
    out_ls = nc.declare_dram_parameter("out_ls", [1, BL], F32, isOutput=True)

    with ExitStack() as ctx:
        tc = ctx.enter_context(tile.TileContext(nc))
        const = ctx.enter_context(tc.tile_pool(name="const", bufs=1))
        state = ctx.enter_context(tc.tile_pool(name="state", bufs=1))
        tmp = ctx.enter_context(tc.tile_pool(name="tmp", bufs=3))
        psum = ctx.enter_context(tc.tile_pool(name="psum", bufs=1, space="PSUM"))

        # --- load params ---
        sb = {}
        for name, shape, dt in PARAM_SHAPES:
            sb[name] = const.tile(shape, dt, tag=name, name=f"sb_{name}")
            nc.sync.dma_start(sb[name][:], prm[name][:])

        # --- persistent state ---
        y16 = state.tile([3, BL * 100], F16, tag="y16")
        nc.gpsimd.memset(y16[:], 0.0)
        us = []
        for l in range(3):
            u = state.tile([128, BL * 100], F16, tag=f"u{l + 1}", name=f"u{l + 1}")
            nc.gpsimd.memset(u[:], 1.0)
            us.append(u)

        def conv(src, wt, taps, m_out, ptag, pbufs=1):
            """src [K,200] padded; wt [K, ntaps*m_out]; returns PSUM [m_out, 128]."""
            h = psum.tile([m_out, BL * 64], F32, tag=ptag, bufs=pbufs, name=ptag)
            for k, (ky, kx) in enumerate(taps):
                rhs = _img(src[:])[:, ky : ky + 8, BL * kx : BL * (kx + 8)]
                nc.tensor.matmul(
                    h[:],
                    wt[:, k * m_out : (k + 1) * m_out],
                    rhs,
                    start=(k == 0),
                    stop=(k == len(taps) - 1),
                )
            return h

        def elu_stage(h, c_t, u_out, pfx):
            """u_out interior <- elu(h + c) + 1 ; h is PSUM [P, 128]."""
            p = h.shape[0]
            ex = tmp.tile([p, BL * 64], F32, tag=f"{pfx}ex", name=f"{pfx}ex")
            nc.scalar.activation(ex[:], h[:], AF.Exp, bias=c_t[:], scale=1.0)
            tp = tmp.tile([p, BL * 64], F32, tag=f"{pfx}tp", name=f"{pfx}tp")
            nc.vector.tensor_scalar(tp[:], h[:], c_t[:], 1.0, ALU.add, ALU.add)
            u_int = _img(u_out[:])[:, 1:9, BL : 9 * BL]
            nc.vector.scalar_tensor_tensor(u_int, ex[:], 1.0, tp[:], ALU.min, ALU.max)

        # --- 22 wavefront steps (all-fp16 matmuls) ---
        # Cone restriction: each layer's activations are FINAL when computed
        # at their own wavefront (they depend only on raster-earlier pixels),
        # so per step we compute h/u only at the <=4 wavefront pixels (N<=8)
        # and cache them; taps of later steps read the cached values.
        ones8 = state.tile([1, 2 * 4], F16, tag="ones8")
        nc.gpsimd.memset(ones8[:], 1.0)
        neg1 = state.tile([128, 1], F32, tag="neg1")
        nc.gpsimd.memset(neg1[:], -1.0)

        def wf_tap(buf, t, i_min, n_i, ky, kx):
            """[P, n_i, 2] view of padded buf at tap (ky,kx) of wavefront t."""
            q0 = 8 * i_min + t + 10 * ky + kx
            return _qb(buf[:])[:, q0 : q0 + 8 * (n_i - 1) + 1 : 8, :]

        def wf_conv(src_buf, wt, dt_row, taps, m_out, ptag, t, i_min, n_i, pbufs=2):
            """PSUM [m_out, 2*n_i] = sum_taps W_tap @ src(tap) (+ optional d row)."""
            W = 2 * n_i
            h = psum.tile([m_out, W], F32, tag=ptag, bufs=pbufs, name=ptag)
            if dt_row is not None:
                nc.tensor.matmul(h[:], dt_row[:], ones8[:, :W], start=True, stop=False)
            for k, (ky, kx) in enumerate(taps):
                rhs = wf_tap(src_buf, t, i_min, n_i, ky, kx)
                nc.tensor.matmul(
                    h[:],
                    wt[:, k * m_out : (k + 1) * m_out],
                    rhs,
                    start=(dt_row is None and k == 0),
                    stop=(k == len(taps) - 1),
                )
            return h

        for t in range(22):
            i_min = max(0, (t - 6) // 2)
            i_max = min(7, t // 2)
            n_i = i_max - i_min + 1
            W = 2 * n_i

            for l, (src_buf, wt, dt_row, taps) in enumerate(
                [
                    (y16, sb["w0"], sb["d1"], TAPS_A),
                    (us[0], sb["w1"], sb["d2"], TAPS_B),
                    (us[1], sb["w2"], sb["d3"], TAPS_B),
                ]
            ):
                # psum h = h_conv + c + 1
                h = wf_conv(src_buf, wt[:], dt_row, taps, 128, f"h{l + 1}", t, i_min, n_i)
                # u[wavefront] = elu(h_conv + c) + 1 = max(psum, min(exp(psum - 1), 1))
                ex = tmp.tile([128, W], F32, tag="aex", name="aex")
                nc.scalar.activation(ex[:], h[:], AF.Exp, bias=neg1[:], scale=1.0)
                q0 = 8 * i_min + t + 11
                u_int = _qb(us[l][:])[:, q0 : q0 + 8 * (n_i - 1) + 1 : 8, :]
                nc.vector.scalar_tensor_tensor(u_int, ex[:], 1.0, h[:], ALU.min, ALU.max)

            o_mu = wf_conv(us[2], sb["w3m"][:], None, TAPS_B, 3, "omu", t, i_min, n_i, pbufs=1)
            o_lv = wf_conv(us[2], sb["w3l"][:], None, TAPS_B, 3, "olv", t, i_min, n_i, pbufs=1)

            # rinv = exp(-0.5*(lv + c4lv)) = 1/exp(logstd)   (ACT, parallel with num)
            rinv = tmp.tile([3, W], F32, tag="rinv", name="rinv")
            nc.scalar.activation(rinv[:], o_lv[:], AF.Exp, bias=sb["nc4lvh"][:], scale=-0.5)
            # num = x_adj - mu_psum  (x_adj already has -c4mu folded in)
            num = tmp.tile([3, W], F32, tag="num", name="num")
            xc = _qb(sb["x_adj"][:])[:, t + 6 * i_min : t + 6 * i_min + 6 * (n_i - 1) + 1 : 6, :]
            nc.vector.scalar_tensor_tensor(num[:], o_mu[:], -1.0, xc, ALU.mult, ALU.add)
            # y16[wavefront] = num * rinv
            ywf = _qb(y16[:])[:, 8 * i_min + t + 11 : 8 * i_min + t + 11 + 8 * (n_i - 1) + 1 : 8, :]
            nvw = num[:].rearrange("p (q b) -> p q b", b=BL, q=n_i)
            rvw = rinv[:].rearrange("p (q b) -> p q b", b=BL, q=n_i)
            nc.vector.tensor_tensor(ywf, nvw, rvw, ALU.mult)

        # --- epilogue 1: fp32 y copy + y output DMA ---
        y32 = state.tile([3, BL * 100], F32, tag="y32")
        nc.vector.tensor_copy(y32[:], y16[:])
        y_int = _img(y32[:])[:, 1:9, BL : 9 * BL]
        nc.sync.dma_start(out_y[:], y_int)

        # --- epilogue 2: fp32 lv-only replay for the logstd sum ---
        ru = []
        for l in range(3):
            u = state.tile([64, BL * 100], F32, tag=f"ru{l + 1}", name=f"ru{l + 1}")
            nc.gpsimd.memset(u[:], 1.0)
            ru.append(u)
        r1 = conv(y32, sb["v0"][:], TAPS_A, 64, "h1", pbufs=2)
        elu_stage(r1, sb["c1l"], ru[0], "r")
        r2 = conv(ru[0], sb["v1"][:], TAPS_B, 64, "h2", pbufs=2)
        elu_stage(r2, sb["c2l"], ru[1], "r")
        r3 = conv(ru[1], sb["v2"][:], TAPS_B, 64, "h3", pbufs=2)
        elu_stage(r3, sb["c3l"], ru[2], "r")
        olvr = conv(ru[2], sb["v3"][:], TAPS_B, 3, "olv")
        lsbuf = state.tile([3, BL * 64], F32, tag="lsbuf")
        nc.vector.tensor_scalar(lsbuf[:], olvr[:], sb["c4lv"][:], 0.5, ALU.add, ALU.mult)
        red = state.tile([3, BL], F32, tag="red")
        lsv = lsbuf[:].rearrange("p (q b) -> p b q", b=BL, q=64)
        nc.vector.tensor_reduce(red[:], lsv, mybir.AxisListType.X, ALU.add)
        ones3 = state.tile([3, 1], F32, tag="ones3")
        nc.gpsimd.memset(ones3[:], 1.0)
        lsps = psum.tile([1, BL], F32, tag="omu")
        nc.tensor.matmul(lsps[:], ones3[:], red[:], start=True, stop=True)
        lso = state.tile([1, BL], F32, tag="lso")
        nc.vector.tensor_copy(lso[:], lsps[:])
        nc.sync.dma_start(out_ls[:], lso[:])

    nc.compile()
    return nc


def prep_params(inputs):
    """Host-side preprocessing of weights (shared across cores)."""
    g = {k: np.asarray(v, np.float32) for k, v in inputs.items()}

    def bd(a, b):
        out = np.zeros((a.shape[0] + b.shape[0], a.shape[1] + b.shape[1]), np.float32)
        out[: a.shape[0], : a.shape[1]] = a
        out[a.shape[0] :, a.shape[1] :] = b
        return out

    p = {}
    p["w0"] = np.concatenate(
        [
            np.concatenate(
                [g["mu_w0"][:, :, ky, kx].T, g["lv_w0"][:, :, ky, kx].T], axis=1
            )
            for ky, kx in TAPS_A
        ],
        axis=1,
    ).astype(np.float16)
    for l, name in ((1, "w1"), (2, "w2")):
        p[name] = np.concatenate(
            [
                bd(g[f"mu_w{l}"][:, :, ky, kx].T, g[f"lv_w{l}"][:, :, ky, kx].T)
                for ky, kx in TAPS_B
            ],
            axis=1,
        ).astype(np.float16)
    p["w3m"] = np.concatenate(
        [
            np.vstack([g["mu_w3"][:, :, ky, kx].T, np.zeros((64, 3), np.float32)])
            for ky, kx in TAPS_B
        ],
        axis=1,
    ).astype(np.float16)
    p["w3l"] = np.concatenate(
        [
            np.vstack([np.zeros((64, 3), np.float32), g["lv_w3"][:, :, ky, kx].T])
            for ky, kx in TAPS_B
        ],
        axis=1,
    ).astype(np.float16)
    # fp32 lv-only replay weights
    p["v0"] = np.ascontiguousarray(
        np.concatenate([g["lv_w0"][:, :, ky, kx].T for ky, kx in TAPS_A], axis=1)
    )
    for l, name in ((1, "v1"), (2, "v2"), (3, "v3")):
        p[name] = np.ascontiguousarray(
            np.concatenate([g[f"lv_w{l}"][:, :, ky, kx].T for ky, kx in TAPS_B], axis=1)
        )
    # bias corrections: layer l>=1 input is u-1 with u-pad=1 -> c_l = b_l - sum(W_l)
    c1 = np.concatenate([g["mu_b0"], g["lv_b0"]])
    c2 = np.concatenate(
        [
            g["mu_b1"] - sum(g["mu_w1"][:, :, ky, kx].sum(1) for ky, kx in TAPS_B),
            g["lv_b1"] - sum(g["lv_w1"][:, :, ky, kx].sum(1) for ky, kx in TAPS_B),
        ]
    )
    c3 = np.concatenate(
        [
            g["mu_b2"] - sum(g["mu_w2"][:, :, ky, kx].sum(1) for ky, kx in TAPS_B),
            g["lv_b2"] - sum(g["lv_w2"][:, :, ky, kx].sum(1) for ky, kx in TAPS_B),
        ]
    )
    p["d1"] = (c1 + 1.0).reshape(1, 128).astype(np.float16)
    p["d2"] = (c2 + 1.0).reshape(1, 128).astype(np.float16)
    p["d3"] = (c3 + 1.0).reshape(1, 128).astype(np.float16)
    p["c1l"] = np.ascontiguousarray(c1[64:].reshape(64, 1))
    p["c2l"] = np.ascontiguousarray(c2[64:].reshape(64, 1))
    p["c3l"] = np.ascontiguousarray(c3[64:].reshape(64, 1))
    c4mu = g["mu_b3"] - sum(g["mu_w3"][:, :, ky, kx].sum(1) for ky, kx in TAPS_B)
    c4lv = g["lv_b3"] - sum(g["lv_w3"][:, :, ky, kx].sum(1) for ky, kx in TAPS_B)
    p["c4lv"] = np.ascontiguousarray(c4lv.reshape(3, 1))
    p["nc4lvh"] = np.ascontiguousarray((-0.5 * c4lv).reshape(3, 1))
    p["_c4mu"] = c4mu
    return p


def make_in_maps(inputs):
    p = prep_params(inputs)
    x = np.asarray(inputs["x"], np.float32)
    shared = {k: np.ascontiguousarray(v) for k, v in p.items() if not k.startswith("_")}
    in_maps = []
    for c in range(N_CORES):
        xs = x[c * BL : (c + 1) * BL]
        # layout (c, h, w, b)
        x_adj = xs.transpose(1, 2, 3, 0).reshape(3, 64 * BL) - p["_c4mu"][:, None]
        m = dict(shared)
        m["x_adj"] = np.ascontiguousarray(x_adj, np.float32)
        in_maps.append(m)
    return in_maps


def kernel(**inputs):
    global _NC_CACHE, LAST_RESULT
    if _NC_CACHE is None:
        _NC_CACHE = build_nc()
    in_maps = make_in_maps(inputs)
    res = run_bass_kernel_spmd(
        _NC_CACHE, in_maps, core_ids=list(range(N_CORES)), trace=TRACE
    )
    LAST_RESULT = res
    ys, lss = [], []
    for c in range(N_CORES):
        ys.append(res.results[c]["out_y"].reshape(3, 8, 8, BL).transpose(3, 0, 1, 2))
        lss.append(res.results[c]["out_ls"].reshape(BL))
    return (
        np.ascontiguousarray(np.concatenate(ys), dtype=np.float32),
        np.ascontiguousarray(np.concatenate(lss), dtype=np.float32),
    )
